# revision 28
# baseline (speedup 1.0000x reference)
"""Trainium2 Bass kernel for nn_AttentionBlock (GroupNorm + 8-head attention
block on [8, 512, 32, 32], residual).

Sharding: pure data-parallel over batch B=8 across the 8 NeuronCores — one
batch element per core, weights replicated, zero collectives.

v3 schedule, built around keeping the Activation engine dense on the 64
[128,1024] exps (the kernel is ACT-bound):
  - GroupNorm is per-channel-tile (each 16-channel group lives inside one
    128-channel tile, so no cross-tile combine): h[t] is ready ~3us after
    x[t] lands, and proj_in starts immediately.
  - proj_in (q/k per head-pair) is interleaved with attention: exp of pair 0
    starts right after qk0+logits0-p0 instead of after all of proj_in.
  - out2 (v @ eT) matmuls of pair hp ride along with the logits matmuls of
    pair hp+1; pair 3's out2 p-steps chase its own exps.
  - softmax denominators: po row 64 (from the vT ones-column) is evicted
    together with attn_u in one [65,1024] bf16 copy; the denom rows take a
    transposing DMA round trip to a [128,2,8] layout where one
    reciprocal_approx_fast costs ~0.2us (DVE time is free-size-based, so
    the single-partition RECIPROCAL of the baseline was 6.5us/head);
    reciprocals return via the inverse DMA and a stride-0 broadcast.
    (reciprocal_approx_fast reads garbage from PSUM — SBUF source only.)
  - b_v is folded into b_out on the host (softmax rows sum to 1, so
    out = w_out@attn_raw + (w_out@b_v + b_out) exactly).
  - GroupNorm's h = a*x+d runs on DVE (tensor_scalar mult+add) to keep ACT
    free for exp; only the vT ones-column is memset.
"""
import sys

sys.path.insert(0, "/opt/trn_rl_repo")

import numpy as np
import ml_dtypes

import concourse.bass as bass
import concourse.bacc as bacc
import concourse.tile as tile
from concourse import mybir
from concourse.bass_utils import run_bass_kernel_spmd

F32 = mybir.dt.float32
BF16 = mybir.dt.bfloat16
FP8 = mybir.dt.float8e4
ADD = mybir.AluOpType.add
MULT = mybir.AluOpType.mult

B, C, H, W = 8, 512, 32, 32
HW = H * W       # 1024
NG = 32          # groups
GS = C // NG     # 16 channels per group
NH = 8           # heads
HD = 64          # head dim
VW = 80          # padded vT cols for fp8 DoubleRow (stride %16)
HID = NH * HD    # 512
NP = NH // 2     # 4 head pairs
EPS = 1e-6
SCALE = 1.0 / float(np.sqrt(HD))  # 0.125
CT = C // 128    # 4 channel partition-tiles
PT = HW // 128   # 8 pixel partition-tiles
GPT = NG // CT   # 8 groups per channel-tile
GN_INV = 1.0 / (GS * HW)          # 1/16384


def build_graph():
    nc = bacc.Bacc("TRN2", num_devices=8)

    x_ext = nc.declare_dram_parameter("x", [C, HW], BF16, isOutput=False)
    w_inT_ext = nc.declare_dram_parameter("w_inT", [C, 3 * HID], BF16, isOutput=False)
    w_outT_ext = nc.declare_dram_parameter("w_outT", [HID, C], BF16, isOutput=False)
    # packed [128, 28] consts: 0:4 gamma, 4:8 beta, 8:16 b_in(q,k),
    # 16:20 b_out_eff, 20:28 gn_sel
    cpack_ext = nc.declare_dram_parameter("cpack", [128, 28], F32, isOutput=False)
    selT_ext = nc.declare_dram_parameter("gn_selT", [GPT, 128], F32, isOutput=False)
    out_ext = nc.declare_dram_parameter("out", [C, HW], F32, isOutput=True)

    rden_dram = nc.dram_tensor("rden_scratch", [NH, HW], F32)

    with tile.TileContext(nc) as tc:
        with (
            tc.tile_pool(name="const", bufs=1) as const,
            tc.tile_pool(name="big", bufs=1) as big,
            tc.tile_pool(name="eT", bufs=1) as eTp,
            tc.tile_pool(name="small", bufs=2) as small,
        ):
            # ---------- loads: x first (GN critical path), then consts,
            # w_inT (needed at first qk matmul), w_outT last ----------
            x_sb = [big.tile([128, HW], BF16, tag=f"x{t}", name=f"x{t}")
                    for t in range(CT)]
            # gpsimd and sync each issue DMA descriptors at ~0.6us per
            # instruction, so loads are few and split across both engines.
            for t in (0, 2):
                nc.gpsimd.dma_start(out=x_sb[t],
                                    in_=x_ext[128 * t:128 * (t + 1), :])
            for t in (1, 3):
                nc.sync.dma_start(out=x_sb[t],
                                  in_=x_ext[128 * t:128 * (t + 1), :])
            cpack_sb = const.tile([128, 28], F32)
            nc.gpsimd.dma_start(out=cpack_sb, in_=cpack_ext[:, :])
            selT_sb = const.tile([GPT, 128], F32)
            nc.gpsimd.dma_start(out=selT_sb, in_=selT_ext[:, :])
            # remaining weight loads issue on gpsimd so sync stays free for
            # the attention-phase scratch DMAs
            gamma_sb = cpack_sb[:, 0:4]
            beta_sb = cpack_sb[:, 4:8]
            b_in_sb = cpack_sb[:, 8:16]
            b_out_sb = cpack_sb[:, 16:20]
            sel_sb = cpack_sb[:, 20:28]
            w_inT_sb = [big.tile([128, 3 * HID], BF16, tag=f"wi{t}", name=f"wi{t}")
                        for t in range(CT)]
            for t in range(CT):
                for blk in range(2):  # q cols then k cols, per tile
                    nc.sync.dma_start(
                        out=w_inT_sb[t][:, HID * blk:HID * (blk + 1)],
                        in_=w_inT_ext[128 * t:128 * (t + 1),
                                      HID * blk:HID * (blk + 1)])
            for t in range(CT):
                nc.sync.dma_start(
                    out=w_inT_sb[t][:, 2 * HID:3 * HID],
                    in_=w_inT_ext[128 * t:128 * (t + 1), 2 * HID:3 * HID])
            w_outT_sb = [big.tile([128, C], BF16, tag=f"wo{t}", name=f"wo{t}")
                         for t in range(CT)]
            for t in range(CT):
                nc.sync.dma_start(out=w_outT_sb[t],
                                  in_=w_outT_ext[128 * t:128 * (t + 1), :])

            # ---------- groupnorm (per channel-tile) + qk0, interleaved ----------
            h_sb = [big.tile([128, HW], BF16, tag=f"h{t}", name=f"h{t}")
                    for t in range(CT)]
            # ---------- fused proj_in + attention ----------
            q_sb = [big.tile([128, HW], BF16, tag=f"q{m}", name=f"q{m}")
                    for m in range(NP)]
            k_sb = [big.tile([128, HW], BF16, tag=f"k{m}", name=f"k{m}")
                    for m in range(NP)]
            vTd_sb = [big.tile([128, 2, NH, VW], FP8, tag=f"vT{P}",
                               name=f"vT{P}") for P in range(PT // 2)]
            attn_sb = [big.tile([128, HW], BF16, tag=f"at{i}", name=f"at{i}")
                       for i in range(NP)]
            po_tiles = {}    # hp -> [po_sub0, po_sub1]
            eT_all = {}      # hp -> [[eT tiles sub0], [sub1]]

            def qk_burst(hp, which):
                """All 8 accumulation matmuls + evict for q or k of pair hp,
                emitted as one slot so the pbig ring is never held open."""
                dest, off, bc = ((q_sb, 0, hp) if which == "q"
                                 else (k_sb, HID, 4 + hp))
                pp = pbig.tile([128, HW], F32, tag="pb",
                               name=f"{which}_acc{hp}")
                for n in range(2):
                    for t in range(CT):
                        nc.tensor.matmul(
                            pp[:, 512 * n:512 * (n + 1)],
                            lhsT=w_inT_sb[t][:, off + 128 * hp:
                                             off + 128 * (hp + 1)],
                            rhs=h_sb[t][:, 512 * n:512 * (n + 1)],
                            start=(t == 0), stop=(t == CT - 1))
                nc.vector.tensor_scalar(
                    out=dest[hp], in0=pp[:, :],
                    scalar1=b_in_sb[:, bc:bc + 1], scalar2=None, op0=ADD)

            def v_one(p):
                """One v tile (pv pool, own PSUM banks), packed fp8 for the
                DoubleRow out2 matmul."""
                if True:
                    pp = pv.tile([128, 512], F32, tag="pv")
                    for t in range(CT):
                        nc.tensor.matmul(
                            pp[:, :],
                            lhsT=h_sb[t][:, 128 * p:128 * (p + 1)],
                            rhs=w_inT_sb[t][:, 2 * HID:3 * HID],
                            start=(t == 0), stop=(t == CT - 1))
                    nc.vector.tensor_copy(
                        out=vTd_sb[p // 2][:, p % 2, :, 0:HD],
                        in_=pp[:, :].rearrange("a (nh c) -> a nh c", nh=NH))

            def out2_step(hp, p):
                """fp8 DoubleRow out2: one accumulation step per p-PAIR
                (fires once both exps of the pair have been emitted)."""
                if p % 2 == 0:
                    return
                P = p // 2
                if P == 0:
                    po_tiles[hp] = [
                        pop.tile([VW, HW], F32, tag="po",
                                 name=f"po{2 * hp + s}") for s in range(2)]
                eTs = eT_all[hp]
                for sub in range(2):
                    head = 2 * hp + sub
                    po_t = po_tiles[hp][sub]
                    for n in range(2):
                        nc.tensor.matmul(
                            po_t[:, 512 * n:512 * (n + 1)],
                            lhsT=vTd_sb[P][:, :, head, :],
                            rhs=eTs[sub][P][:, :, 512 * n:512 * (n + 1)],
                            perf_mode=mybir.MatmulPerfMode.DoubleRow,
                            start=(P == 0), stop=(P == PT // 2 - 1))

            def emit_logits_exp(hp, out2_of=None, fillers=None):
                eTs = [[eTp.tile([128, 2, HW], FP8, bufs=2,
                                 tag=f"eT{sub}_{P}",
                                 name=f"eT{hp}_{sub}_{P}")
                        for P in range(PT // 2)] for sub in range(2)]
                eT_all[hp] = eTs
                for p in range(PT):
                    pls = []
                    for sub in range(2):
                        lo = 64 * sub
                        pl = pbig.tile([128, HW], F32, tag="pb",
                                       name=f"pl{hp}_{sub}_{p}")
                        for n in range(2):
                            nc.tensor.matmul(
                                pl[:, 512 * n:512 * (n + 1)],
                                lhsT=k_sb[hp][lo:lo + 64, 128 * p:128 * (p + 1)],
                                rhs=q_sb[hp][lo:lo + 64, 512 * n:512 * (n + 1)],
                                start=True, stop=True)
                        pls.append(pl)
                    for sub in range(2):
                        nc.scalar.activation(
                            out=eTs[sub][p // 2][:, p % 2, :],
                            in_=pls[sub][:, :],
                            func=mybir.ActivationFunctionType.Exp,
                            bias=expb_sb[:, :], scale=SCALE)
                    if out2_of is not None:
                        out2_step(out2_of, p)
                    for f in (fillers or {}).get(p, []):
                        f()

            pair_state = {}

            def finish_den(hp, pe_bcast=False):
                """Start the denominator chain ASAP: den row hop to
                partition base 0 (reciprocal_approx_fast is wrong at
                nonzero base), approx, then either write reciprocals to
                DRAM (steady-state pairs) or broadcast them on the PE into
                freed po banks (tail pair, lowest latency)."""
                eT_all.pop(hp)
                pos = po_tiles.pop(hp)
                # rows 0 and 32 so both can serve as matmul rhs
                den2 = small.tile([33, HW], F32, tag="den2", bufs=2,
                                  name=f"den2_{hp}")
                rr2 = small.tile([33, HW], F32, tag="rr2", bufs=2,
                                 name=f"rr2_{hp}")
                au_s = []
                dma_eng = nc.gpsimd if pe_bcast else nc.sync
                rrows = []
                for sub in range(2):
                    rrow = small.tile([HD + 1, HW], F32, tag="rrow",
                                      bufs=2, name=f"rrow{2 * hp + sub}")
                    nc.vector.tensor_copy(out=rrow[HD:HD + 1, :],
                                          in_=pos[sub][HD:HD + 1, :])
                    dma_eng.dma_start(out=den2[32 * sub:32 * sub + 1, :],
                                      in_=rrow[HD:HD + 1, :])
                    rrows.append(rrow)
                for sub in range(2):
                    au = small.tile([HD, HW], BF16, tag="attnu", bufs=4,
                                    name=f"attnu{2 * hp + sub}")
                    nc.vector.tensor_copy(out=au, in_=pos[sub][0:HD, :])
                    au_s.append(au)
                nc.vector.reciprocal_approx_fast(out=rr2, in_=den2)
                if pe_bcast:
                    bbs = []
                    for sub in range(2):
                        bb = pop.tile([VW, HW], F32, tag="po",
                                      name=f"bb{hp}_{sub}")
                        for n in range(2):
                            nc.tensor.matmul(
                                bb[0:HD, 512 * n:512 * (n + 1)],
                                lhsT=ones1_sb[32 * sub:32 * sub + 1, :],
                                rhs=rr2[32 * sub:32 * sub + 1,
                                        512 * n:512 * (n + 1)],
                                start=True, stop=True)
                        bbs.append(bb)
                    pair_state[hp] = (au_s, bbs)
                else:
                    for sub in range(2):
                        nc.sync.dma_start(
                            out=rden_dram[2 * hp + sub:2 * hp + sub + 1, :],
                            in_=rr2[32 * sub:32 * sub + 1, :])
                    pair_state[hp] = (au_s, None)

            def finish_mul(hp):
                """Normalize: emitted a pair-window after finish_den so the
                DMA round trip never head-of-line blocks the DVE queue."""
                au_s, bbs = pair_state.pop(hp)
                for sub in range(2):
                    head = 2 * hp + sub
                    if bbs is not None:
                        rb = bbs[sub][0:HD, :]
                    else:
                        rbt = small.tile([HD, HW], F32, tag="rb", bufs=2,
                                         name=f"rb{head}")
                        bcast_ap = bass.AP(
                            tensor=rden_dram[:, :].tensor,
                            offset=head * HW,
                            ap=[[0, HD], [1, HW]])
                        nc.sync.dma_start(out=rbt, in_=bcast_ap)
                        rb = rbt[:, :]
                    if sub == 0:
                        nc.vector.tensor_mul(attn_sb[hp][0:HD, :],
                                             au_s[sub][:, :], rb)
                    else:
                        tmp2 = small.tile([HD, HW], BF16, tag="atmp2",
                                          bufs=2, name=f"atmp2{head}")
                        nc.vector.tensor_mul(tmp2[:, :], au_s[sub][:, :],
                                             rb)
                        (nc.gpsimd if bbs is not None else
                         nc.sync).dma_start(out=attn_sb[hp][HD:128, :],
                                            in_=tmp2)

            with tc.tile_pool(name="pbig", bufs=2, space="PSUM") as pbig:
                # GN per tile + qk0 accumulation steps ride along so the
                # first exp fires as soon as x3 lands.
                ppq0 = pbig.tile([128, HW], F32, tag="pb", name="q_acc0")
                ppk0 = pbig.tile([128, HW], F32, tag="pb", name="k_acc0")
                with tc.tile_pool(name="ps_gn", bufs=2, space="PSUM") as ps_gn:
                    eps_sb = small.tile([GPT, 1], F32, tag="eps_c", bufs=1)
                    nc.gpsimd.memset(eps_sb, float(EPS))
                    sq_scratch = small.tile([128, HW], BF16, tag="sqs",
                                            bufs=1)
                    for t in range(CT):
                        st = small.tile([128, 2], F32, tag=f"st{t}", bufs=1,
                                        name=f"st{t}")
                        # plain sum on gpsimd, sum of squares on DVE (2x
                        # bf16 mode) — parallel engines, one x pass each
                        nc.vector.reduce_sum(st[:, 0:1], x_sb[t][:, :],
                                             axis=mybir.AxisListType.X)
                        nc.vector.scalar_tensor_tensor(
                            out=sq_scratch, in0=x_sb[t][:, :], scalar=1.0,
                            in1=x_sb[t][:, :],
                            op0=mybir.AluOpType.bypass, op1=MULT,
                            accum_out=st[:, 1:2])
                        gpsum = ps_gn.tile([GPT, 2], F32, tag="gps")
                        nc.tensor.matmul(gpsum[:, :], lhsT=sel_sb[:, :],
                                         rhs=st[:, :], start=True, stop=True)
                        # grp cols: 0 rstd, 1 mean*rstd, 2 mean, 3 E[x^2]
                        grp = small.tile([GPT, 4], F32, tag="grp", bufs=2,
                                         name=f"grp{t}")
                        nc.vector.tensor_scalar_mul(grp[:, 2:4],
                                                    gpsum[:, 0:2], GN_INV)
                        nc.gpsimd.tensor_mul(grp[:, 0:1], grp[:, 2:3],
                                             grp[:, 2:3])
                        nc.gpsimd.tensor_sub(grp[:, 0:1], grp[:, 3:4],
                                             grp[:, 0:1])
                        nc.scalar.activation(
                            out=grp[:, 0:1], in_=grp[:, 0:1],
                            func=mybir.ActivationFunctionType.Sqrt,
                            bias=eps_sb[:, :], scale=1.0)
                        nc.vector.reciprocal(out=grp[:, 0:1], in_=grp[:, 0:1])
                        nc.gpsimd.tensor_mul(grp[:, 1:2], grp[:, 2:3],
                                             grp[:, 0:1])
                        epsum = ps_gn.tile([128, 2], F32, tag="eps")
                        nc.tensor.matmul(epsum[:, :], lhsT=selT_sb[:, :],
                                         rhs=grp[:, 0:2], start=True,
                                         stop=True)
                        ga = small.tile([128, 1], F32, tag=f"ga{t}", bufs=1,
                                        name=f"ga{t}")
                        gd = small.tile([128, 1], F32, tag=f"gd{t}", bufs=1,
                                        name=f"gd{t}")
                        nc.vector.tensor_mul(ga[:, :], gamma_sb[:, t:t + 1],
                                             epsum[:, 0:1])
                        nc.vector.tensor_mul(gd[:, :], gamma_sb[:, t:t + 1],
                                             epsum[:, 1:2])
                        nc.gpsimd.tensor_sub(gd[:, :], beta_sb[:, t:t + 1],
                                             gd[:, :])
                        nc.vector.tensor_scalar(
                            out=h_sb[t], in0=x_sb[t][:, :],
                            scalar1=ga[:, :], scalar2=gd[:, :],
                            op0=MULT, op1=ADD)
                        for n in range(2):
                            nc.tensor.matmul(
                                ppq0[:, 512 * n:512 * (n + 1)],
                                lhsT=w_inT_sb[t][:, 0:128],
                                rhs=h_sb[t][:, 512 * n:512 * (n + 1)],
                                start=(t == 0), stop=(t == CT - 1))
                            nc.tensor.matmul(
                                ppk0[:, 512 * n:512 * (n + 1)],
                                lhsT=w_inT_sb[t][:, HID:HID + 128],
                                rhs=h_sb[t][:, 512 * n:512 * (n + 1)],
                                start=(t == 0), stop=(t == CT - 1))
                nc.vector.tensor_scalar(
                    out=q_sb[0], in0=ppq0[:, :],
                    scalar1=b_in_sb[:, 0:1], scalar2=None, op0=ADD)
                nc.vector.tensor_scalar(
                    out=k_sb[0], in0=ppk0[:, :],
                    scalar1=b_in_sb[:, 4:5], scalar2=None, op0=ADD)
                for P in range(PT // 2):
                    nc.vector.memset(vTd_sb[P][:, :, :, HD:VW], 0.0)
                    nc.vector.memset(vTd_sb[P][:, :, :, HD:HD + 1], 1.0)
                ones1_sb = const.tile([33, HD], F32)
                nc.gpsimd.memset(ones1_sb, 1.0)
                # constant logit shift: keeps exp within fp8e4 range;
                # softmax is shift-invariant so the ratio is exact.
                expb_sb = const.tile([128, 1], F32)
                nc.gpsimd.memset(expb_sb, -4.0)
                with tc.tile_pool(name="pv", bufs=2, space="PSUM") as pv:
                    emit_logits_exp(0, fillers={
                        0: [lambda: v_one(0)],
                        1: [lambda: v_one(1)],
                        2: [lambda: qk_burst(1, "q")],
                        3: [lambda: v_one(2)],
                        4: [lambda: v_one(3)],
                        5: [lambda: qk_burst(1, "k")],
                        6: [lambda: v_one(4), lambda: v_one(5)],
                        7: [lambda: v_one(6), lambda: v_one(7)],
                    })
                with tc.tile_pool(name="po", bufs=2, space="PSUM") as pop:
                    emit_logits_exp(1, out2_of=0, fillers={
                        2: [lambda: qk_burst(2, "q")],
                        5: [lambda: qk_burst(2, "k")],
                    })
                    finish_den(0)
                    emit_logits_exp(2, out2_of=1, fillers={
                        2: [lambda: qk_burst(3, "q")],
                        5: [lambda: qk_burst(3, "k")],
                    })
                    finish_den(1)
                    finish_mul(0)
                    emit_logits_exp(3, out2_of=2)
                    finish_den(2)
                    finish_mul(1)
                    for p in range(PT):
                        out2_step(3, p)
                    finish_den(3, pe_bcast=True)
                    finish_mul(2)
                    finish_mul(3)

            # ---------- proj_out + bias + residual ----------
            # k-steps over pairs 0-2 are emitted first for every chunk so
            # they execute while pair 3 is still in flight; only the
            # 8 final k3 steps + stt + output DMA remain in the tail.
            with tc.tile_pool(name="ps_pout", bufs=8, space="PSUM") as ps_pout:
                pps = {}
                for m in range(CT):
                    for n in range(2):
                        pp = ps_pout.tile([128, 512], F32, tag="pp",
                                          name=f"po_{m}_{n}")
                        pps[(m, n)] = pp
                        for t in range(CT - 1):
                            nc.tensor.matmul(
                                pp[:, :],
                                lhsT=w_outT_sb[t][:, 128 * m:128 * (m + 1)],
                                rhs=attn_sb[t][:, 512 * n:512 * (n + 1)],
                                start=(t == 0), stop=False)
                for m in range(CT):
                    for n in range(2):
                        pp = pps[(m, n)]
                        t = CT - 1
                        nc.tensor.matmul(
                            pp[:, :],
                            lhsT=w_outT_sb[t][:, 128 * m:128 * (m + 1)],
                            rhs=attn_sb[t][:, 512 * n:512 * (n + 1)],
                            start=False, stop=True)
                        o_sb = small.tile([128, 512], F32, tag="osb", bufs=4)
                        nc.vector.scalar_tensor_tensor(
                            out=o_sb, in0=pp[:, :],
                            scalar=b_out_sb[:, m:m + 1],
                            in1=x_sb[m][:, 512 * n:512 * (n + 1)],
                            op0=ADD, op1=ADD)
                        for hh in range(2):
                            eng = nc.gpsimd if (2 * m + n + hh) % 2 else nc.sync
                            eng.dma_start(
                                out=out_ext[128 * m:128 * (m + 1),
                                            512 * n + 256 * hh:
                                            512 * n + 256 * (hh + 1)],
                                in_=o_sb[:, 256 * hh:256 * (hh + 1)])
    return nc


def _install_ntff_hook():
    """The agent image's antenv lacks axon_hooks; synthesize it so
    run_bass_kernel_spmd(trace=True) can reach the NTFF profiler."""
    import types
    if "antenv.axon_hooks" in sys.modules:
        return
    mod = types.ModuleType("antenv.axon_hooks")
    mod._hook = None

    def set_axon_ntff_profile_hook(hook):
        mod._hook = hook

    def get_axon_ntff_profile_hook():
        return mod._hook

    mod.set_axon_ntff_profile_hook = set_axon_ntff_profile_hook
    mod.get_axon_ntff_profile_hook = get_axon_ntff_profile_hook
    sys.modules["antenv.axon_hooks"] = mod
    try:
        from trn_agent_boot.trn_boot import _ntff_profile_via_ctypes
        hook = _ntff_profile_via_ctypes("/opt/axon/libaxon_pjrt.so")
        if hook is not None:
            set_axon_ntff_profile_hook(hook)
    except Exception as e:  # degrade to no tracing
        print("ntff hook setup failed:", e)


_COMPILED = None


def _get_compiled():
    global _COMPILED
    if _COMPILED is None:
        nc = build_graph()
        nc.compile()
        _COMPILED = nc
    return _COMPILED


def _make_consts():
    # within any 128-channel tile, partition p belongs to local group p//16
    sel = np.zeros((128, GPT), dtype=np.float32)
    selT = np.zeros((GPT, 128), dtype=np.float32)
    for p in range(128):
        sel[p, p // GS] = 1.0
        selT[p // GS, p] = 1.0
    return sel, selT


def _pm(v, cols):
    """[cols*128] vector -> partition-major [128, cols]."""
    return np.ascontiguousarray(v.reshape(cols, 128).T)


def kernel(x, gamma, beta, w_in, b_in, w_out, b_out, _trace=False):
    x = np.asarray(x, dtype=np.float32)
    gamma = np.asarray(gamma, dtype=np.float32)
    beta = np.asarray(beta, dtype=np.float32)
    w_in = np.asarray(w_in, dtype=np.float32)
    b_in = np.asarray(b_in, dtype=np.float32)
    w_out = np.asarray(w_out, dtype=np.float32)
    b_out = np.asarray(b_out, dtype=np.float32)

    w_inT = np.ascontiguousarray(w_in.T).astype(ml_dtypes.bfloat16)
    w_outT = np.ascontiguousarray(w_out.T).astype(ml_dtypes.bfloat16)
    sel, selT = _make_consts()
    # fold v-bias through proj_out: softmax rows sum to 1, so the attention
    # output is attn_raw + b_v exactly; w_out @ b_v + b_out replaces b_out.
    b_v = b_in[2 * HID:3 * HID]
    b_out_eff = b_out + w_out.astype(np.float64) @ b_v.astype(np.float64)
    b_out_eff = b_out_eff.astype(np.float32)
    cpack = np.zeros((128, 28), dtype=np.float32)
    cpack[:, 0:4] = _pm(gamma, CT)
    cpack[:, 4:8] = _pm(beta, CT)
    cpack[:, 8:16] = _pm(b_in[0:2 * HID], 8)
    cpack[:, 16:20] = _pm(b_out_eff, CT)
    cpack[:, 20:28] = sel
    common = {
        "w_inT": w_inT,
        "w_outT": w_outT,
        "cpack": cpack,
        "gn_selT": selT,
    }
    in_maps = []
    for b in range(B):
        m = dict(common)
        m["x"] = np.ascontiguousarray(x[b].reshape(C, HW)).astype(
            ml_dtypes.bfloat16)
        in_maps.append(m)

    if _trace:
        _install_ntff_hook()
    nc = _get_compiled()
    res = run_bass_kernel_spmd(nc, in_maps, core_ids=list(range(B)),
                               trace=_trace)
    out = np.stack([np.asarray(res.results[b]["out"]).reshape(C, H, W)
                    for b in range(B)])
    if _trace:
        return out, res
    return out


if __name__ == "__main__":
    rng = np.random.default_rng(0)
    inputs = {
        "x": rng.standard_normal((B, C, H, W), dtype=np.float32),
        "gamma": np.ones(C, dtype=np.float32),
        "beta": np.zeros(C, dtype=np.float32),
        "w_in": (rng.standard_normal((3 * HID, C), dtype=np.float32)
                 / np.sqrt(C)),
        "b_in": np.zeros(3 * HID, dtype=np.float32),
        "w_out": (rng.standard_normal((C, HID), dtype=np.float32)
                  / np.sqrt(HID)),
        "b_out": np.zeros(C, dtype=np.float32),
    }
    out = kernel(**inputs)
    print("kernel ran, out shape", out.shape)
